# revision 21
# baseline (speedup 1.0000x reference)
"""CWS (Chinese word segmentation) greedy-agenda model kernel for trn2.

Strategy: the expensive, fully-parallel part of the model — the
per-word-length reset gate and the composition projection — depends only on
(char_id, word_length), not on the position.  The device computes the proj
TABLE over the (padded) vocabulary, sharded 768 char ids per core across 8
NeuronCores (embarrassingly parallel, parameters replicated, no
collectives), and the host gathers table[chars] per position.

Numerics are exact fp32 end-to-end: the recurrence argmax has score
margins down to 1.6e-6 across 32640 decisions, so any reduced-precision
table (bf16: 170 flips, tf32: ~1e-4 table error) fails.  MM1 therefore
runs as a 3-pass f32r (tf32) decomposition — z = Rhi.T@ehi + Rhi.T@elo +
Rlo.T@ehi accumulated in fp32 PSUM, identical to fp32 within 7e-8 but
3 cyc/row instead of fp32's 4 — and MM2 stays true fp32 (its moving
operand is device-computed; splitting it costs more than the PE saves).

Device schedule (one pass per core, no loops):
  - a bf16 dummy matmul on a DVE-memset scratch warms the PE p-state so
    every real matmul runs at the full 2.4 GHz
  - weights ship pre-split (tf32 hi/lo); the char-embedding shard ships
    as plain fp32 once and DVE splits it on device (DMA transfers all
    serialize on one resource, so bytes moved = serial time)
  - per-(i,w) sigmoid on ACT (bias differs per w), per-w muls on DVE,
    per-pair tanh (shared bias), per-pair output DMAs with the last pair
    split per-w (and the final w split 256+128) for a short tail

The remaining recurrence (score -> argmax -> LSTM -> buffer shift) is a
tiny, strictly-sequential chain over T=256 steps, vectorized over B on
host using the precomputed word tensors.  If the device path fails the
kernel falls back to a numerically-identical host computation.
"""

import contextlib

import numpy as np

B, T, L, DC, DW, H, V = 128, 256, 4, 128, 128, 256, 6000
NEG = -1e30
N_CORES = 8
VPAD = 6144                # vocab padded to 8 * 768
P = VPAD // N_CORES        # 768 vocab rows per core
CHUNK = 384
NCHUNK = P // CHUNK        # 2
N_DUMMY = 1


def _sigmoid(x):
    out = np.empty_like(x)
    np.negative(x, out=out)
    np.exp(out, out=out)
    out += 1.0
    np.reciprocal(out, out=out)
    return out


def _tf32(x):
    xv = np.ascontiguousarray(x, np.float32).view(np.int32)
    return ((xv + 0x1000) & ~0x1FFF).astype(np.int32).view(np.float32)


def _proj_host(chars, char_emb, reset_W, reset_b, com_W, com_b):
    emb = char_emb[chars]                       # [B, T, DC]
    flat = emb.reshape(B * T, DC)
    proj = np.empty((L, B * T, DW), np.float32)
    for w in range(L):
        g = _sigmoid(flat @ reset_W[w] + reset_b[w])
        g *= flat
        proj[w] = np.tanh(g @ com_W + com_b)
    return proj.reshape(L, B, T, DW)


def _build_bass(n_dummy=N_DUMMY):
    """Raw Bass SPMD program (explicit semaphores; one condition per wait —
    this walrus build rejects instructions carrying multiple attached
    waits, so TileContext is not usable here)."""
    import concourse.bass as bass
    from concourse import mybir

    nc = bass.Bass()
    f32 = mybir.dt.float32
    f32r = mybir.dt.float32r
    bf16 = mybir.dt.bfloat16
    AF = mybir.ActivationFunctionType

    parWh_in = nc.dram_tensor("parWh", [DC, L * DC], f32r, kind="ExternalInput")
    parWl_in = nc.dram_tensor("parWl", [DC, L * DC], f32r, kind="ExternalInput")
    parB_in = nc.dram_tensor("parB", [DC, L + 1], f32, kind="ExternalInput")
    parC_in = nc.dram_tensor("parC", [DC, DW], f32, kind="ExternalInput")
    emb_in = nc.dram_tensor("embT", [DC, P], f32, kind="ExternalInput")
    # dw-major output layout: [DW, L, P] matches the SBUF element order
    # (partition=dw outermost), so pair-wide out-DMAs use natural
    # descending-stride APs (w-major [L, DW, P] would need a permuted AP,
    # which this runtime's DGE can't handle)
    proj_out = nc.dram_tensor("projT", [DW, L, P], f32, kind="ExternalOutput")

    ctx = contextlib.ExitStack()
    with ctx:
        parWh = ctx.enter_context(nc.sbuf_tensor([DC, L * DC], f32r))
        parWl = ctx.enter_context(nc.sbuf_tensor([DC, L * DC], f32r))
        parB = ctx.enter_context(nc.sbuf_tensor([DC, L + 1], f32))
        parC = ctx.enter_context(nc.sbuf_tensor([DC, DW], f32))
        emb = ctx.enter_context(nc.sbuf_tensor([DC, NCHUNK, CHUNK], f32))
        ehi = ctx.enter_context(nc.sbuf_tensor([DC, NCHUNK, CHUNK], f32r))
        elo = ctx.enter_context(nc.sbuf_tensor([DC, NCHUNK, CHUNK], f32r))
        g = ctx.enter_context(nc.sbuf_tensor([DC, 8, CHUNK], f32))
        pj = ctx.enter_context(nc.sbuf_tensor([DW, 8, CHUNK], f32))
        scr = ctx.enter_context(nc.sbuf_tensor([DC, CHUNK], f32))
        warm = ctx.enter_context(nc.sbuf_tensor([1, 2], f32))
        ps = ctx.enter_context(nc.psum_tensor([DC, 8, 512], f32))
        s_e0 = ctx.enter_context(nc.semaphore())
        s_wh = ctx.enter_context(nc.semaphore())
        s_wl = ctx.enter_context(nc.semaphore())
        s_e1 = ctx.enter_context(nc.semaphore())
        s_b = ctx.enter_context(nc.semaphore())
        s_c = ctx.enter_context(nc.semaphore())
        dma_out = ctx.enter_context(nc.semaphore())
        pe1 = ctx.enter_context(nc.semaphore())
        pe2 = ctx.enter_context(nc.semaphore())
        act1 = ctx.enter_context(nc.semaphore())
        act2 = ctx.enter_context(nc.semaphore())
        dve = ctx.enter_context(nc.semaphore())
        dvs = ctx.enter_context(nc.semaphore())
        scrdone = ctx.enter_context(nc.semaphore())
        blk = ctx.enter_context(nc.Block())

        # single DMA lane (transfers serialize on one resource anyway);
        # order chosen so each input lands just before its first consumer
        @blk.sync
        def _(sync):
            sync.dma_start(
                out=emb[:, 0, :], in_=emb_in[:, bass.ts(0, CHUNK)]
            ).then_inc(s_e0, 16)
            sync.dma_start(out=parWh[:, :], in_=parWh_in[:, :]).then_inc(s_wh, 16)
            sync.dma_start(out=parWl[:, :], in_=parWl_in[:, :]).then_inc(s_wl, 16)
            sync.dma_start(
                out=emb[:, 1, :], in_=emb_in[:, bass.ts(1, CHUNK)]
            ).then_inc(s_e1, 16)
            sync.dma_start(out=parB[:, :], in_=parB_in[:, :]).then_inc(s_b, 16)
            sync.dma_start(out=parC[:, :], in_=parC_in[:, :]).then_inc(s_c, 16)
            # pair-wide outputs in the dw-major layout; q1 goes via the
            # Pool/SWDGE lane (below) to keep this HWDGE chain short
            for q in (0, 2):
                i, p = divmod(q, 2)
                sync.wait_ge(act2, q + 1)
                sync.dma_start(
                    out=proj_out[:, 2 * p : 2 * p + 2, bass.ts(i, CHUNK)],
                    in_=pj[:, 4 * i + 2 * p : 4 * i + 2 * p + 2, :],
                ).then_inc(dma_out, 16)
            sync.wait_ge(act2, 4)
            sync.dma_start(
                out=proj_out[:, 2, bass.ts(1, CHUNK)], in_=pj[:, 6, :]
            ).then_inc(dma_out, 16)
            sync.wait_ge(act2, 5)
            sync.dma_start(
                out=proj_out[:, 3, CHUNK : CHUNK + 256], in_=pj[:, 7, :256]
            ).then_inc(dma_out, 16)
            sync.wait_ge(act2, 6)
            sync.dma_start(
                out=proj_out[:, 3, CHUNK + 256 :], in_=pj[:, 7, 256:]
            ).then_inc(dma_out, 16)


        # q1's and k6's output configs run on the Pool/SWDGE lane, off the
        # shared HWDGE, so the SP chain stays short for the tail DMAs
        @blk.gpsimd
        def _(gp):
            gp.wait_ge(act2, 2)
            gp.dma_start(
                out=proj_out[:, 2:4, bass.ts(0, CHUNK)],
                in_=pj[:, 2:4, :],
            ).then_inc(dma_out, 16)


        @blk.tensor
        def _(tensor):
            if n_dummy:
                tensor.wait_ge(scrdone, 1)
                for _d in range(n_dummy):
                    # scr is f32; bitcast halves the element size, so take
                    # half-width slices for bf16 operands
                    nc.tensor.matmul(
                        ps[:, 7, :CHUNK],
                        scr[:, : DC // 2].bitcast(bf16),
                        scr[:, : CHUNK // 2].bitcast(bf16),
                        start=True,
                        stop=True,
                    )
            # MM1: 3 f32r passes per (i, w) into bank 4i+w.  Pass order
            # a = Rhi.ehi (start), c = Rhi.elo (mid), b = Rlo.ehi (stop)
            # matches DMA/split arrival order.
            for i in range(NCHUNK):
                tensor.wait_ge(dvs, 2 * i + 1)          # ehi(i)
                if i == 0:
                    tensor.wait_ge(s_wh, 16)            # parWh
                for w in range(L):
                    nc.tensor.matmul(
                        ps[:, 4 * i + w, :CHUNK],
                        parWh[:, bass.ts(w, DC)],
                        ehi[:, i, :],
                        start=True,
                        stop=False,
                    ).then_inc(pe1, 1)
                tensor.wait_ge(dvs, 2 * i + 2)          # elo(i)
                for w in range(L):
                    nc.tensor.matmul(
                        ps[:, 4 * i + w, :CHUNK],
                        parWh[:, bass.ts(w, DC)],
                        elo[:, i, :],
                        start=False,
                        stop=False,
                    ).then_inc(pe1, 1)
                if i == 0:
                    tensor.wait_ge(s_wl, 16)            # parWl
                for w in range(L):
                    nc.tensor.matmul(
                        ps[:, 4 * i + w, :CHUNK],
                        parWl[:, bass.ts(w, DC)],
                        ehi[:, i, :],
                        start=False,
                        stop=True,
                    ).then_inc(pe1, 1)
            # MM2: true fp32, bank k reused after sigma(k) read it
            tensor.wait_ge(s_c, 16)                     # parC
            for k in range(8):
                tensor.wait_ge(dve, k + 1)
                nc.tensor.matmul(
                    ps[:, k, :CHUNK], parC[:, :], g[:, k, :],
                    start=True, stop=True,
                ).then_inc(pe2, 1)

        @blk.scalar
        def _(scalar):
            # warm the sigmoid/tanh ACT tables off the critical path; AP
            # biases (garbage values are fine) avoid const-pool memsets in
            # the preamble
            nc.scalar.activation(
                warm[:, 0:1], warm[:, 1:2], AF.Sigmoid, bias=warm[:, 0:1]
            )
            nc.scalar.activation(
                warm[:, 0:1], warm[:, 1:2], AF.Tanh, bias=warm[:, 0:1]
            )
            scalar.wait_ge(s_b, 16)  # parB
            for k in range(8):
                i, w = divmod(k, L)
                scalar.wait_ge(pe1, 12 * i + 8 + w + 1)  # b-pass (i,w) done
                nc.scalar.activation(
                    g[:, k, :], ps[:, k, :CHUNK], AF.Sigmoid,
                    bias=parB[:, w : w + 1],
                ).then_inc(act1, 1)
            for q in range(3):
                scalar.wait_ge(pe2, 2 * q + 2)
                nc.scalar.activation(
                    pj[:, 2 * q : 2 * q + 2, :],
                    ps[:, 2 * q : 2 * q + 2, :CHUNK],
                    AF.Tanh,
                    bias=parB[:, L : L + 1],
                ).then_inc(act2, 1)
            scalar.wait_ge(pe2, 7)
            nc.scalar.activation(
                pj[:, 6, :], ps[:, 6, :CHUNK], AF.Tanh, bias=parB[:, L : L + 1]
            ).then_inc(act2, 1)
            scalar.wait_ge(pe2, 8)
            nc.scalar.activation(
                pj[:, 7, :256], ps[:, 7, :256], AF.Tanh, bias=parB[:, L : L + 1]
            ).then_inc(act2, 1)
            nc.scalar.activation(
                pj[:, 7, 256:], ps[:, 7, 256:CHUNK], AF.Tanh,
                bias=parB[:, L : L + 1],
            ).then_inc(act2, 1)
            # NOTE: do NOT issue the tail DMAs from this queue without an
            # act2 wait — a same-queue DMA only orders against the tanh at
            # the sequencer, not against its completion (observed as
            # nondeterministic stale reads on HW)

        @blk.vector
        def _(vector):
            nc.vector.memset(scr[:, :].bitcast(mybir.dt.uint32), 0.0).then_inc(
                scrdone, 1
            )
            # both tf32 splits FIRST (they gate PE), then the gate muls
            for i in range(NCHUNK):
                vector.wait_ge(s_e0 if i == 0 else s_e1, 16)
                nc.vector.tensor_copy(ehi[:, i, :], emb[:, i, :]).then_inc(dvs, 1)
                nc.vector.tensor_sub(
                    elo[:, i, :], emb[:, i, :], ehi[:, i, :].bitcast(f32)
                ).then_inc(dvs, 1)
            for k in range(8):
                vector.wait_ge(act1, k + 1)
                nc.vector.tensor_mul(
                    g[:, k, :], g[:, k, :], emb[:, k // L, :]
                ).then_inc(dve, 1)
    return nc


def _try_device_proj(chars, char_emb, reset_W, reset_b, com_W, com_b,
                     trace=False):
    try:
        from concourse.bass_utils import run_bass_kernel_spmd

        nc = _build_bass()
        # Vocab-sharded: core c computes the proj table for char ids
        # [c*P, (c+1)*P).  Parameters replicated, tf32 hi/lo split on host.
        emb_pad = np.zeros((VPAD, DC), np.float32)
        emb_pad[:V] = char_emb
        parW = np.ascontiguousarray(
            reset_W.transpose(1, 0, 2).reshape(DC, L * DC), np.float32
        )
        parWh = _tf32(parW)
        parWl = _tf32(parW - parWh)
        parB = np.ascontiguousarray(
            np.concatenate([reset_b.T, com_b[:, None]], axis=1), np.float32
        )
        parC = np.ascontiguousarray(com_W, np.float32)
        in_maps = []
        for c in range(N_CORES):
            shard = emb_pad[c * P : (c + 1) * P]            # [P, DC]
            in_maps.append({
                "parWh": parWh,
                "parWl": parWl,
                "parB": parB,
                "parC": parC,
                "embT": np.ascontiguousarray(shard.T, np.float32),
            })
        res = run_bass_kernel_spmd(nc, in_maps, core_ids=list(range(N_CORES)),
                                   trace=trace)
        # device emits [DW, L, P] per core; assemble to [L, VPAD, DW]
        table = np.concatenate(
            [res.results[c]["projT"] for c in range(N_CORES)], axis=2
        ).transpose(1, 2, 0)                                # [L, VPAD, DW]
        proj = np.ascontiguousarray(
            table[:, chars.reshape(-1), :].reshape(L, B, T, DW))
        if trace:
            print(f"HW exec time: {res.exec_time_ns} ns")
        return proj
    except Exception:  # pragma: no cover
        import traceback
        traceback.print_exc()
        print("[kernel] device path failed; host fallback")
        return None


def _word_from_proj(proj):
    """word[b, t, w, :] = mean_{c<=w} proj[w, b, t-c, :]."""
    word = np.zeros((B, T, L, DW), np.float32)
    for w in range(L):
        acc = proj[w].copy()
        for c in range(1, w + 1):
            acc[:, c:] += proj[w][:, :-c]
        word[:, :, w, :] = acc / np.float32(w + 1)
    return word


def kernel(chars, char_emb, reset_W, reset_b, com_W, com_b, lstm_kernel,
           lstm_bias, pred_W, pred_b, score_U, bos):
    chars = np.asarray(chars)
    char_emb = np.asarray(char_emb, np.float32)
    reset_W = np.asarray(reset_W, np.float32)
    reset_b = np.asarray(reset_b, np.float32)
    com_W = np.asarray(com_W, np.float32)
    com_b = np.asarray(com_b, np.float32)
    lstm_kernel = np.asarray(lstm_kernel, np.float32)
    lstm_bias = np.asarray(lstm_bias, np.float32)
    pred_W = np.asarray(pred_W, np.float32)
    pred_b = np.asarray(pred_b, np.float32)
    score_U = np.asarray(score_U, np.float32)
    bos = np.asarray(bos, np.float32)

    proj = _try_device_proj(chars, char_emb, reset_W, reset_b, com_W, com_b)
    if proj is None:
        proj = _proj_host(chars, char_emb, reset_W, reset_b, com_W, com_b)
    word = _word_from_proj(proj)                # [B, T, L, DW]

    # ---- sequential agenda recurrence (host, vectorized over B) ----
    Kx = lstm_kernel[:DW]
    Kh = lstm_kernel[DW:]

    def lstm(x, c, h):
        z = x @ Kx + h @ Kh + lstm_bias
        i = z[:, :H]; j = z[:, H:2*H]; f = z[:, 2*H:3*H]; o = z[:, 3*H:]
        ncell = c * _sigmoid(f) + _sigmoid(i) * np.tanh(j)
        nh = np.tanh(ncell) * _sigmoid(o)
        return ncell, nh

    c0 = np.zeros((B, H), np.float32)
    h0 = np.zeros((B, H), np.float32)
    x0 = np.broadcast_to(bos, (B, DW))
    c1, h1 = lstm(x0, c0, h0)
    pred0 = np.tanh(h1 @ pred_W + pred_b)
    buf_pred = np.repeat(pred0[:, None, :], L, axis=1)
    buf_c = np.repeat(c1[:, None, :], L, axis=1)
    buf_h = np.repeat(h1[:, None, :], L, axis=1)

    wlens = np.arange(1, L + 1)
    bidx = np.arange(B)
    scores_out = np.empty((T, B), np.float32)
    wl_out = np.empty((T, B), np.int32)
    for t in range(T):
        wt = word[:, t]                          # [B, L, DW]
        score = np.einsum("ble,ble->bl", buf_pred + score_U, wt).astype(np.float32)
        score = np.where((wlens <= t + 1)[None, :], score, np.float32(NEG))
        best = np.argmax(score, axis=1)
        word_b = wt[bidx, best]
        c_prev = buf_c[bidx, best]
        h_prev = buf_h[bidx, best]
        ncell, nh = lstm(word_b, c_prev, h_prev)
        npred = np.tanh(nh @ pred_W + pred_b)
        buf_pred = np.concatenate([npred[:, None], buf_pred[:, :-1]], axis=1)
        buf_c = np.concatenate([ncell[:, None], buf_c[:, :-1]], axis=1)
        buf_h = np.concatenate([nh[:, None], buf_h[:, :-1]], axis=1)
        scores_out[t] = score[bidx, best]
        wl_out[t] = best + 1

    return scores_out.T.copy(), wl_out.T.copy()


if __name__ == "__main__":
    d = dict(np.load("/tmp/inputs.npz"))
    s, w = kernel(**d)
    print(s.shape, w.shape)
